# revision 17
# baseline (speedup 1.0000x reference)
"""CovaBlock kernel for 8 trn2 NeuronCores.

reference computation:
  cova[w] = covariance of support class w over its 8*32*32 = 8192 pixels  [16,128,128]
  qn[b]   = x1[b] with each channel row L2-normalized over the 1024 pixels
  sim[b, w, i] = qn[b,:,i]^T @ cova[w] @ qn[b,:,i]   -> [256, 16*1024]

Plan:
  launch 1: shard the 16 classes over 8 cores (2 each) -> cova on device
  host:     cholesky(cova) = L L^T for the even classes (tiny)
  launch 2: shard the 256 queries over 8 cores (32 each). Per query:
    - qn scaled by 2^6 on the Pool engine (bf16)
    - 16 main matmuls U_w = mats_w^T qn (bf16, 2x N=512 halves)
    - V_w = U_w^2 on ACT (even classes, mats=chol) or U_w*qn on DVE
      (odd classes, mats=cova), written as fp8e4m3 into per-pair packs
    - column-sum reduction: 8 fp8 DoubleRow matmuls (2 classes/stream via
      one-hot [128,2,16] stationaries) accumulate all 16 rows into one
      [16,1024] PSUM tile
    - stage halves on ACT/DVE apply the 2^-12 descale, one DMA out
"""

import os

import numpy as np

import concourse.bass as bass
import concourse.tile as tile
from concourse import bacc, mybir
from concourse.bass_utils import run_bass_kernel_spmd
from concourse.masks import make_identity

F32 = mybir.dt.float32
F32R = mybir.dt.float32r
BF16 = mybir.dt.bfloat16
F8 = mybir.dt.float8e4

N_CORES = 8
B, C, HW = 256, 128, 1024          # x1: [B, C, 32, 32]
W, S = 16, 8                       # x2: [W, S, C, 32, 32]
NS = S * HW                        # samples per class = 8192
BS = B // N_CORES                  # 32 queries per core
WS = W // N_CORES                  # 2 classes per core

Q_SCALE = 64.0                     # 2^6 on qn  -> V scaled by 2^12
DESCALE = 1.0 / (Q_SCALE * Q_SCALE)

_CACHE = {}


def _build_cova_nc():
    """Per-core: x2 pair [2, 8, 128, 1024] f32 -> cova pair [2, 128, 128] f32."""
    nc = bacc.Bacc("TRN2", target_bir_lowering=False, debug=False,
                   num_devices=N_CORES)
    x2p = nc.dram_tensor("x2p", [WS, S, C, HW], F32, kind="ExternalInput").ap()
    cov = nc.dram_tensor("cova_pair", [WS, C, C], F32, kind="ExternalOutput").ap()

    inv_nm1 = 1.0 / (NS - 1)
    # mean scale so that mmT comes out as N/(N-1) * m m^T directly:
    # m'' = msum * sqrt(N/(N-1)) / N
    mscale = float(np.sqrt(NS / (NS - 1.0)) / NS)

    with tile.TileContext(nc) as tc:
        NK = HW // 128  # 128-column transpose chunks per shot

        with (
            tc.tile_pool(name="consts", bufs=1) as consts,
            tc.tile_pool(name="raw", bufs=3) as raw,
            tc.tile_pool(name="xb", bufs=9) as xbp,
            tc.tile_pool(name="xt", bufs=3) as xtp,
            tc.tile_pool(name="small", bufs=4) as small,
            tc.tile_pool(name="scratch", bufs=2) as scratch,
            tc.tile_pool(name="cout", bufs=2) as cout,
            tc.tile_pool(name="pt", bufs=2, space="PSUM") as pt,
            tc.tile_pool(name="pe", bufs=2, space="PSUM") as pe,
            tc.tile_pool(name="pm", bufs=1, space="PSUM") as pm,
            tc.tile_pool(name="pmr", bufs=1, space="PSUM") as pmr,
        ):
            ident = consts.tile([128, 128], BF16)
            make_identity(nc, ident)
            ident32 = consts.tile([128, 128], F32)
            make_identity(nc, ident32)

            for w in range(WS):
                # load shots (split across the SP HWDGE queue and the gpsimd
                # SWDGE queue so transfers overlap), convert to bf16 on ACT
                # while accumulating the per-channel sum into msum columns
                xb16 = []
                msum = small.tile([C, S], F32)
                for s in range(S):
                    xr = raw.tile([C, HW], F32, tag="raw")
                    eng = nc.default_dma_engine if s % 2 == 0 else nc.gpsimd
                    eng.dma_start(out=xr, in_=x2p[w, s])
                    xb = xbp.tile([C, HW], BF16, tag=f"xb{s}")
                    if (w * S + s) % 5 == 4:
                        # spill some converts to DVE (fused accumulate via
                        # tensor_scalar) -- ACT is the launch bottleneck
                        nc.vector.tensor_scalar(
                            out=xb, in0=xr, scalar1=1.0, scalar2=None,
                            op0=mybir.AluOpType.mult,
                            accum_out=msum[:, s : s + 1])
                    else:
                        nc.scalar.activation(
                            xb, xr, mybir.ActivationFunctionType.Copy,
                            accum_out=msum[:, s : s + 1])
                    xb16.append(xb)

                # m'' = (sum_s msum[:, s]) * mscale, as fp32 [C, 1]
                mtot = small.tile([C, 1], F32)
                nc.vector.reduce_sum(mtot, msum, axis=mybir.AxisListType.X)
                mpp = small.tile([C, 1], F32)
                nc.scalar.mul(mpp, mtot, mscale)

                # E = sum over 64 chunks of X_chunk X_chunk^T  (bf16, fp32
                # acc). Transposes land batched in one psum bank so a single
                # DVE copy stages all 8 chunks of a shot.
                e_ps = pe.tile([C, C], F32, tag="E")
                for s in range(S):
                    xt_ps = pt.tile([128, NK, C], BF16, tag="xt_ps")
                    for k in range(NK):
                        chunk = xb16[s][:, k * 128 : (k + 1) * 128]
                        nc.tensor.transpose(xt_ps[:, k, :], chunk, ident)
                    xt = xtp.tile([128, NK, C], BF16, tag="xt")
                    nc.vector.tensor_copy(xt, xt_ps)
                    for k in range(NK):
                        nc.tensor.matmul(e_ps, xt[:, k, :], xt[:, k, :],
                                         start=(s == 0 and k == 0),
                                         stop=(s == S - 1 and k == NK - 1))

                # mmT = m'' m''^T via K=1 matmul; needs m'' as a [1, C] row
                mrow_ps = pmr.tile([1, C], F32, tag="mrow")
                nc.tensor.transpose(mrow_ps, mpp, ident32)
                mrow = small.tile([1, C], F32)
                nc.vector.tensor_copy(mrow, mrow_ps)
                mmT_ps = pm.tile([C, C], F32, tag="mmT")
                nc.tensor.matmul(mmT_ps, mrow, mrow, start=True, stop=True)
                mmT = scratch.tile([C, C], F32, tag="mmT_sb")
                nc.vector.tensor_copy(mmT, mmT_ps)

                # C_w = E * 1/(N-1) - mmT
                cw = cout.tile([C, C], F32, tag="cw")
                nc.vector.scalar_tensor_tensor(
                    out=cw, in0=e_ps, scalar=inv_nm1, in1=mmT,
                    op0=mybir.AluOpType.mult, op1=mybir.AluOpType.subtract)
                nc.default_dma_engine.dma_start(out=cov[w], in_=cw)

    nc.compile()
    return nc


# classes on the ACT square path (mats = cholesky factor); odd classes use
# the DVE multiply path (mats = cova). Strict alternation keeps both engines
# fed every other class — any run of same-engine classes stalls the other.
ACT_SET = frozenset(range(0, W, 2))


def _build_sim_nc():
    """Per-core: x1 shard [32, 128, 1024] + mats [16, 128, 128] bf16 ->
    sim shard [32, 16, 1024]."""
    nc = bacc.Bacc("TRN2", target_bir_lowering=False, debug=False,
                   num_devices=N_CORES)
    x1s = nc.dram_tensor("x1s", [BS, C, HW], F32, kind="ExternalInput").ap()
    mats = nc.dram_tensor("mats", [W, C, C], BF16, kind="ExternalInput").ap()
    out = nc.dram_tensor("sim", [BS, W, HW], F32, kind="ExternalOutput").ap()

    NP = W // 2  # DoubleRow class pairs
    H = HW // 2  # 512-column halves

    with tile.TileContext(nc) as tc:
        with (
            tc.tile_pool(name="consts", bufs=1) as consts,
            tc.tile_pool(name="xb", bufs=4) as xbp,
            tc.tile_pool(name="sq", bufs=3) as sqp,
            tc.tile_pool(name="qn", bufs=3) as qnp,
            tc.tile_pool(name="vpk", bufs=2) as vpkp,
            tc.tile_pool(name="stg", bufs=2) as stgp,
            tc.tile_pool(name="small", bufs=9) as small,
            tc.tile_pool(name="pU", bufs=3, space="PSUM") as pU,
            tc.tile_pool(name="pacc", bufs=1, space="PSUM") as pacc,
        ):
            # one-hot DoubleRow reduction stationaries: pair p computes
            # rows (2p, 2p+1) of the sim accumulator from the packed
            # [128, 2, 512] fp8 V slice.
            drw = consts.tile([C, NP, 2, W], F8)
            nc.gpsimd.memset(drw, 0.0)
            for p in range(NP):
                for i in range(2):
                    w = 2 * p + i
                    nc.gpsimd.memset(drw[:, p, i, w : w + 1], 1.0)

            xbt = {}

            def emit_xb_dma(b):
                if b >= BS:
                    return
                xbt[b] = xbp.tile([C, HW], F32, tag="xb", name=f"xb{b}")
                nc.default_dma_engine.dma_start(out=xbt[b], in_=x1s[b])

            emit_xb_dma(0)

            # stationary matrices, already bf16 from the host: one DMA
            # (2048 x 256B descriptors) so the ACT sequencer stays free
            mat = consts.tile([C, W, C], BF16)
            nc.default_dma_engine.dma_start(
                out=mat, in_=mats.rearrange("w c d -> c w d"))

            emit_xb_dma(1)
            emit_xb_dma(2)

            qnt = {}

            nrmt = {}

            def emit_chain_act(b):
                """ACT half of the norm chain for query b: n2 -> nrm."""
                if b >= BS:
                    return
                sq = sqp.tile([C, HW], BF16, tag="sq")
                n2 = small.tile([C, 1], F32, tag="n2")
                nc.scalar.activation(sq, xbt[b],
                                     mybir.ActivationFunctionType.Square,
                                     accum_out=n2)
                nrmt[b] = small.tile([C, 1], F32, tag="nrm", name=f"nrm{b}")
                nc.scalar.sqrt(nrmt[b], n2)

            def emit_chain_rest(b):
                """DVE recip + Pool qn. Emitted mid-query so the DVE FIFO
                head never blocks on the cross-engine sqrt dependency."""
                if b >= BS:
                    return
                rinv = small.tile([C, 1], F32, tag="rinv")
                nc.vector.reciprocal(rinv, nrmt.pop(b))
                qnt[b] = qnp.tile([C, HW], BF16, tag="qn", name=f"qn{b}")
                nc.gpsimd.tensor_scalar(out=qnt[b], in0=xbt[b], scalar1=rinv,
                                        scalar2=Q_SCALE,
                                        op0=mybir.AluOpType.mult,
                                        op1=mybir.AluOpType.mult)

            def emit_stage(b, acc):
                """Stage + descale + output DMA for a finished query."""
                stg = stgp.tile([W, HW], F32, tag="stg")
                nc.scalar.activation(stg[:, 0:H], acc[:, 0:H],
                                     mybir.ActivationFunctionType.Copy,
                                     scale=DESCALE)
                nc.vector.tensor_scalar_mul(stg[:, H:HW], acc[:, H:HW], DESCALE)
                nc.default_dma_engine.dma_start(out=out[b], in_=stg)

            # two-query-ahead norm chains keep qn off every critical path
            emit_chain_act(0)
            emit_chain_rest(0)
            emit_chain_act(1)
            emit_chain_rest(1)

            def emit_drs(b, vpk):
                """8 DoubleRow matmuls accumulate all 16 class sums into one
                [16, 1024] psum tile (rows = classes)."""
                acc = pacc.tile([W, HW], F32, tag="acc")
                for h in range(2):
                    cols = slice(h * H, (h + 1) * H)
                    for p in range(NP):
                        nc.tensor.matmul(
                            acc[:, cols], drw[:, p, :, :], vpk[:, p, :, cols],
                            start=(p == 0), stop=(p == NP - 1),
                            perf_mode=mybir.MatmulPerfMode.DoubleRow)
                return acc

            prev = None  # (b, vpk) of the previous query
            for b in range(BS):
                emit_xb_dma(b + 3)
                emit_chain_act(b + 2)

                qn = qnt.pop(b)
                xbt.pop(b)
                vpk = vpkp.tile([C, NP, 2, HW], F8, tag="vpk")
                for w in range(W):
                    u_ps = pU.tile([C, HW], F32, tag="u")
                    for h in range(2):
                        cols = slice(h * H, (h + 1) * H)
                        nc.tensor.matmul(u_ps[:, cols], mat[:, w, :],
                                         qn[:, cols], start=True, stop=True)
                    vslot = vpk[:, w // 2, w % 2, :]
                    if w in ACT_SET:
                        # cholesky path: V = U^2 (ACT)
                        nc.scalar.square(vslot, u_ps)
                    else:
                        # direct path: V = U * qn (DVE)
                        nc.vector.tensor_mul(vslot, u_ps, qn)
                    if w == 2 and prev is not None:
                        # previous query's reduction + stage, deferred past
                        # the first mains of this query so the engines have
                        # fresh U tiles when the old query's V work drains
                        pb, pvpk = prev
                        pacc_t = emit_drs(pb, pvpk)
                        emit_stage(pb, pacc_t)
                    if w == 3:
                        emit_chain_rest(b + 2)

                prev = (b, vpk)

            acc = emit_drs(*prev)
            emit_stage(prev[0], acc)

    nc.compile()
    return nc


def kernel(x1: np.ndarray, x2: np.ndarray) -> np.ndarray:
    x1 = np.ascontiguousarray(np.asarray(x1, dtype=np.float32)).reshape(B, C, HW)
    x2 = np.ascontiguousarray(np.asarray(x2, dtype=np.float32)).reshape(W, S, C, HW)
    core_ids = list(range(N_CORES))

    if "cova" not in _CACHE:
        _CACHE["cova"] = _build_cova_nc()
    cova_in = [{"x2p": np.ascontiguousarray(x2[WS * k : WS * (k + 1)])}
               for k in range(N_CORES)]
    res1 = run_bass_kernel_spmd(_CACHE["cova"], cova_in, core_ids)
    cova = np.concatenate([res1.results[k]["cova_pair"] for k in range(N_CORES)], 0)

    # ACT_SET classes go through the cholesky/ACT-square path
    import ml_dtypes

    act_idx = sorted(ACT_SET)
    mats = cova.copy()
    mats[act_idx] = np.linalg.cholesky(
        cova[act_idx].astype(np.float64)).astype(np.float32)
    mats = np.ascontiguousarray(mats.astype(ml_dtypes.bfloat16))

    if "sim" not in _CACHE:
        _CACHE["sim"] = _build_sim_nc()
    sim_in = [{"x1s": np.ascontiguousarray(x1[BS * k : BS * (k + 1)]),
               "mats": mats} for k in range(N_CORES)]
    res2 = run_bass_kernel_spmd(_CACHE["sim"], sim_in, core_ids)
    sim = np.concatenate([res2.results[k]["sim"] for k in range(N_CORES)], 0)
    return sim.reshape(B, W * HW)


# revision 18
# speedup vs baseline: 1.0076x; 1.0076x over previous
"""CovaBlock kernel for 8 trn2 NeuronCores.

reference computation:
  cova[w] = covariance of support class w over its 8*32*32 = 8192 pixels  [16,128,128]
  qn[b]   = x1[b] with each channel row L2-normalized over the 1024 pixels
  sim[b, w, i] = qn[b,:,i]^T @ cova[w] @ qn[b,:,i]   -> [256, 16*1024]

Plan:
  launch 1: shard the 16 classes over 8 cores (2 each) -> cova on device
  host:     cholesky(cova) = L L^T for the even classes (tiny)
  launch 2: shard the 256 queries over 8 cores (32 each). Per query:
    - qn scaled by 2^6 on the Pool engine (bf16)
    - 16 main matmuls U_w = mats_w^T qn (bf16, 2x N=512 halves)
    - V_w = U_w^2 on ACT (even classes, mats=chol) or U_w*qn on DVE
      (odd classes, mats=cova), written as fp8e4m3 into per-pair packs
    - column-sum reduction: 8 fp8 DoubleRow matmuls (2 classes/stream via
      one-hot [128,2,16] stationaries) accumulate all 16 rows into one
      [16,1024] PSUM tile
    - stage halves on ACT/DVE apply the 2^-12 descale, one DMA out
"""

import os

import numpy as np

import concourse.bass as bass
import concourse.tile as tile
from concourse import bacc, mybir
from concourse.bass_utils import run_bass_kernel_spmd
from concourse.masks import make_identity

F32 = mybir.dt.float32
F32R = mybir.dt.float32r
BF16 = mybir.dt.bfloat16
F8 = mybir.dt.float8e4

N_CORES = 8
B, C, HW = 256, 128, 1024          # x1: [B, C, 32, 32]
W, S = 16, 8                       # x2: [W, S, C, 32, 32]
NS = S * HW                        # samples per class = 8192
BS = B // N_CORES                  # 32 queries per core
WS = W // N_CORES                  # 2 classes per core

Q_SCALE = 64.0                     # 2^6 on qn  -> V scaled by 2^12
DESCALE = 1.0 / (Q_SCALE * Q_SCALE)

_CACHE = {}


def _build_cova_nc():
    """Per-core: x2 pair [2, 8, 128, 1024] f32 -> cova pair [2, 128, 128] f32."""
    nc = bacc.Bacc("TRN2", target_bir_lowering=False, debug=False,
                   num_devices=N_CORES)
    x2p = nc.dram_tensor("x2p", [WS, S, C, HW], F32, kind="ExternalInput").ap()
    cov = nc.dram_tensor("cova_pair", [WS, C, C], F32, kind="ExternalOutput").ap()

    inv_nm1 = 1.0 / (NS - 1)
    # mean scale so that mmT comes out as N/(N-1) * m m^T directly:
    # m'' = msum * sqrt(N/(N-1)) / N
    mscale = float(np.sqrt(NS / (NS - 1.0)) / NS)

    with tile.TileContext(nc) as tc:
        NK = HW // 128  # 128-column transpose chunks per shot

        with (
            tc.tile_pool(name="consts", bufs=1) as consts,
            tc.tile_pool(name="raw", bufs=3) as raw,
            tc.tile_pool(name="xb", bufs=9) as xbp,
            tc.tile_pool(name="xt", bufs=3) as xtp,
            tc.tile_pool(name="small", bufs=4) as small,
            tc.tile_pool(name="scratch", bufs=2) as scratch,
            tc.tile_pool(name="cout", bufs=2) as cout,
            tc.tile_pool(name="pt", bufs=2, space="PSUM") as pt,
            tc.tile_pool(name="pe", bufs=2, space="PSUM") as pe,
            tc.tile_pool(name="pm", bufs=1, space="PSUM") as pm,
            tc.tile_pool(name="pmr", bufs=1, space="PSUM") as pmr,
        ):
            ident = consts.tile([128, 128], BF16)
            make_identity(nc, ident)
            ident32 = consts.tile([128, 128], F32)
            make_identity(nc, ident32)

            for w in range(WS):
                # load shots (split across the SP HWDGE queue and the gpsimd
                # SWDGE queue so transfers overlap), convert to bf16 on ACT
                # while accumulating the per-channel sum into msum columns
                xb16 = []
                msum = small.tile([C, S], F32)
                for s in range(S):
                    xr = raw.tile([C, HW], F32, tag="raw")
                    eng = nc.default_dma_engine if s % 2 == 0 else nc.gpsimd
                    eng.dma_start(out=xr, in_=x2p[w, s])
                    xb = xbp.tile([C, HW], BF16, tag=f"xb{s}")
                    nc.scalar.activation(xb, xr, mybir.ActivationFunctionType.Copy,
                                         accum_out=msum[:, s : s + 1])
                    xb16.append(xb)

                # m'' = (sum_s msum[:, s]) * mscale, as fp32 [C, 1]
                mtot = small.tile([C, 1], F32)
                nc.vector.reduce_sum(mtot, msum, axis=mybir.AxisListType.X)
                mpp = small.tile([C, 1], F32)
                nc.scalar.mul(mpp, mtot, mscale)

                # E = sum over 64 chunks of X_chunk X_chunk^T  (bf16, fp32
                # acc). Transposes land batched in one psum bank so a single
                # DVE copy stages all 8 chunks of a shot.
                e_ps = pe.tile([C, C], F32, tag="E")
                for s in range(S):
                    xt_ps = pt.tile([128, NK, C], BF16, tag="xt_ps")
                    for k in range(NK):
                        chunk = xb16[s][:, k * 128 : (k + 1) * 128]
                        nc.tensor.transpose(xt_ps[:, k, :], chunk, ident)
                    xt = xtp.tile([128, NK, C], BF16, tag="xt")
                    nc.vector.tensor_copy(xt, xt_ps)
                    for k in range(NK):
                        nc.tensor.matmul(e_ps, xt[:, k, :], xt[:, k, :],
                                         start=(s == 0 and k == 0),
                                         stop=(s == S - 1 and k == NK - 1))

                # mmT = m'' m''^T via K=1 matmul; needs m'' as a [1, C] row
                mrow_ps = pmr.tile([1, C], F32, tag="mrow")
                nc.tensor.transpose(mrow_ps, mpp, ident32)
                mrow = small.tile([1, C], F32)
                nc.vector.tensor_copy(mrow, mrow_ps)
                mmT_ps = pm.tile([C, C], F32, tag="mmT")
                nc.tensor.matmul(mmT_ps, mrow, mrow, start=True, stop=True)
                mmT = scratch.tile([C, C], F32, tag="mmT_sb")
                nc.vector.tensor_copy(mmT, mmT_ps)

                # C_w = E * 1/(N-1) - mmT
                cw = cout.tile([C, C], F32, tag="cw")
                nc.vector.scalar_tensor_tensor(
                    out=cw, in0=e_ps, scalar=inv_nm1, in1=mmT,
                    op0=mybir.AluOpType.mult, op1=mybir.AluOpType.subtract)
                nc.default_dma_engine.dma_start(out=cov[w], in_=cw)

    nc.compile()
    return nc


# classes on the ACT square path (mats = cholesky factor); odd classes use
# the DVE multiply path (mats = cova). Strict alternation keeps both engines
# fed every other class — any run of same-engine classes stalls the other.
ACT_SET = frozenset(range(0, W, 2))


def _build_sim_nc():
    """Per-core: x1 shard [32, 128, 1024] + mats [16, 128, 128] bf16 ->
    sim shard [32, 16, 1024]."""
    nc = bacc.Bacc("TRN2", target_bir_lowering=False, debug=False,
                   num_devices=N_CORES)
    x1s = nc.dram_tensor("x1s", [BS, C, HW], F32, kind="ExternalInput").ap()
    mats = nc.dram_tensor("mats", [W, C, C], BF16, kind="ExternalInput").ap()
    out = nc.dram_tensor("sim", [BS, W, HW], F32, kind="ExternalOutput").ap()

    NP = W // 2  # DoubleRow class pairs
    H = HW // 2  # 512-column halves

    with tile.TileContext(nc) as tc:
        with (
            tc.tile_pool(name="consts", bufs=1) as consts,
            tc.tile_pool(name="xb", bufs=4) as xbp,
            tc.tile_pool(name="sq", bufs=3) as sqp,
            tc.tile_pool(name="qn", bufs=3) as qnp,
            tc.tile_pool(name="vpk", bufs=2) as vpkp,
            tc.tile_pool(name="stg", bufs=2) as stgp,
            tc.tile_pool(name="small", bufs=9) as small,
            tc.tile_pool(name="pU", bufs=3, space="PSUM") as pU,
            tc.tile_pool(name="pacc", bufs=1, space="PSUM") as pacc,
        ):
            # one-hot DoubleRow reduction stationaries: pair p computes
            # rows (2p, 2p+1) of the sim accumulator from the packed
            # [128, 2, 512] fp8 V slice.
            drw = consts.tile([C, NP, 2, W], F8)
            nc.gpsimd.memset(drw, 0.0)
            for p in range(NP):
                for i in range(2):
                    w = 2 * p + i
                    nc.gpsimd.memset(drw[:, p, i, w : w + 1], 1.0)

            xbt = {}

            def emit_xb_dma(b):
                if b >= BS:
                    return
                xbt[b] = xbp.tile([C, HW], F32, tag="xb", name=f"xb{b}")
                nc.default_dma_engine.dma_start(out=xbt[b], in_=x1s[b])

            emit_xb_dma(0)

            # stationary matrices, already bf16 from the host: one DMA
            # (2048 x 256B descriptors) so the ACT sequencer stays free
            mat = consts.tile([C, W, C], BF16)
            nc.default_dma_engine.dma_start(
                out=mat, in_=mats.rearrange("w c d -> c w d"))

            emit_xb_dma(1)
            emit_xb_dma(2)

            qnt = {}

            nrmt = {}

            def emit_chain_act(b):
                """ACT half of the norm chain for query b: n2 -> nrm."""
                if b >= BS:
                    return
                sq = sqp.tile([C, HW], BF16, tag="sq")
                n2 = small.tile([C, 1], F32, tag="n2")
                nc.scalar.activation(sq, xbt[b],
                                     mybir.ActivationFunctionType.Square,
                                     accum_out=n2)
                nrmt[b] = small.tile([C, 1], F32, tag="nrm", name=f"nrm{b}")
                nc.scalar.sqrt(nrmt[b], n2)

            def emit_chain_rest(b):
                """DVE recip + Pool qn. Emitted mid-query so the DVE FIFO
                head never blocks on the cross-engine sqrt dependency."""
                if b >= BS:
                    return
                rinv = small.tile([C, 1], F32, tag="rinv")
                nc.vector.reciprocal(rinv, nrmt.pop(b))
                qnt[b] = qnp.tile([C, HW], BF16, tag="qn", name=f"qn{b}")
                nc.gpsimd.tensor_scalar(out=qnt[b], in0=xbt[b], scalar1=rinv,
                                        scalar2=Q_SCALE,
                                        op0=mybir.AluOpType.mult,
                                        op1=mybir.AluOpType.mult)

            def emit_stage(b, acc):
                """Stage + descale + output DMA for a finished query."""
                stg = stgp.tile([W, HW], F32, tag="stg")
                nc.scalar.activation(stg[:, 0:H], acc[:, 0:H],
                                     mybir.ActivationFunctionType.Copy,
                                     scale=DESCALE)
                nc.vector.tensor_scalar_mul(stg[:, H:HW], acc[:, H:HW], DESCALE)
                nc.default_dma_engine.dma_start(out=out[b], in_=stg)

            # two-query-ahead norm chains keep qn off every critical path
            emit_chain_act(0)
            emit_chain_rest(0)
            emit_chain_act(1)
            emit_chain_rest(1)

            def emit_drs(b, vpk):
                """8 DoubleRow matmuls accumulate all 16 class sums into one
                [16, 1024] psum tile (rows = classes)."""
                acc = pacc.tile([W, HW], F32, tag="acc")
                for h in range(2):
                    cols = slice(h * H, (h + 1) * H)
                    for p in range(NP):
                        nc.tensor.matmul(
                            acc[:, cols], drw[:, p, :, :], vpk[:, p, :, cols],
                            start=(p == 0), stop=(p == NP - 1),
                            perf_mode=mybir.MatmulPerfMode.DoubleRow)
                return acc

            prev = None  # (b, vpk) of the previous query
            for b in range(BS):
                emit_xb_dma(b + 3)
                emit_chain_act(b + 2)

                qn = qnt.pop(b)
                xbt.pop(b)
                vpk = vpkp.tile([C, NP, 2, HW], F8, tag="vpk")
                for w in range(W):
                    u_ps = pU.tile([C, HW], F32, tag="u")
                    for h in range(2):
                        cols = slice(h * H, (h + 1) * H)
                        nc.tensor.matmul(u_ps[:, cols], mat[:, w, :],
                                         qn[:, cols], start=True, stop=True)
                    vslot = vpk[:, w // 2, w % 2, :]
                    if w in ACT_SET:
                        # cholesky path: V = U^2 (ACT)
                        nc.scalar.square(vslot, u_ps)
                    else:
                        # direct path: V = U * qn (DVE)
                        nc.vector.tensor_mul(vslot, u_ps, qn)
                    if w == 2 and prev is not None:
                        # previous query's reduction + stage, deferred past
                        # the first mains of this query so the engines have
                        # fresh U tiles when the old query's V work drains
                        pb, pvpk = prev
                        pacc_t = emit_drs(pb, pvpk)
                        emit_stage(pb, pacc_t)
                    if w == 3:
                        emit_chain_rest(b + 2)

                prev = (b, vpk)

            acc = emit_drs(*prev)
            emit_stage(prev[0], acc)

    nc.compile()
    return nc


def kernel(x1: np.ndarray, x2: np.ndarray) -> np.ndarray:
    x1 = np.ascontiguousarray(np.asarray(x1, dtype=np.float32)).reshape(B, C, HW)
    x2 = np.ascontiguousarray(np.asarray(x2, dtype=np.float32)).reshape(W, S, C, HW)
    core_ids = list(range(N_CORES))

    if "cova" not in _CACHE:
        _CACHE["cova"] = _build_cova_nc()
    cova_in = [{"x2p": np.ascontiguousarray(x2[WS * k : WS * (k + 1)])}
               for k in range(N_CORES)]
    res1 = run_bass_kernel_spmd(_CACHE["cova"], cova_in, core_ids)
    cova = np.concatenate([res1.results[k]["cova_pair"] for k in range(N_CORES)], 0)

    # ACT_SET classes go through the cholesky/ACT-square path
    import ml_dtypes

    act_idx = sorted(ACT_SET)
    mats = cova.copy()
    mats[act_idx] = np.linalg.cholesky(
        cova[act_idx].astype(np.float64)).astype(np.float32)
    mats = np.ascontiguousarray(mats.astype(ml_dtypes.bfloat16))

    if "sim" not in _CACHE:
        _CACHE["sim"] = _build_sim_nc()
    sim_in = [{"x1s": np.ascontiguousarray(x1[BS * k : BS * (k + 1)]),
               "mats": mats} for k in range(N_CORES)]
    res2 = run_bass_kernel_spmd(_CACHE["sim"], sim_in, core_ids)
    sim = np.concatenate([res2.results[k]["sim"] for k in range(N_CORES)], 0)
    return sim.reshape(B, W * HW)


# revision 20
# speedup vs baseline: 1.0428x; 1.0349x over previous
"""CovaBlock kernel for 8 trn2 NeuronCores.

reference computation:
  cova[w] = covariance of support class w over its 8*32*32 = 8192 pixels  [16,128,128]
  qn[b]   = x1[b] with each channel row L2-normalized over the 1024 pixels
  sim[b, w, i] = qn[b,:,i]^T @ cova[w] @ qn[b,:,i]   -> [256, 16*1024]

Plan:
  launch 1: shard the 16 classes over 8 cores (2 each) -> cova on device
  host:     cholesky(cova) = L L^T for the even classes (tiny)
  launch 2: shard the 256 queries over 8 cores (32 each). Per query:
    - qn scaled by 2^6 on the Pool engine (bf16)
    - 16 main matmuls U_w = mats_w^T qn (bf16, 2x N=512 halves)
    - V_w = U_w^2 on ACT (even classes, mats=chol) or U_w*qn on DVE
      (odd classes, mats=cova), written as fp8e4m3 into per-pair packs
    - column-sum reduction: 8 fp8 DoubleRow matmuls (2 classes/stream via
      one-hot [128,2,16] stationaries) accumulate all 16 rows into one
      [16,1024] PSUM tile
    - stage halves on ACT/DVE apply the 2^-12 descale, one DMA out
"""

import os

import numpy as np

import concourse.bass as bass
import concourse.tile as tile
from concourse import bacc, mybir
from concourse.bass_utils import run_bass_kernel_spmd
from concourse.masks import make_identity

F32 = mybir.dt.float32
F32R = mybir.dt.float32r
BF16 = mybir.dt.bfloat16
F8 = mybir.dt.float8e4

N_CORES = 8
B, C, HW = 256, 128, 1024          # x1: [B, C, 32, 32]
W, S = 16, 8                       # x2: [W, S, C, 32, 32]
NS = S * HW                        # samples per class = 8192
BS = B // N_CORES                  # 32 queries per core
WS = W // N_CORES                  # 2 classes per core

Q_SCALE = 64.0                     # 2^6 on qn  -> V scaled by 2^12
DESCALE = 1.0 / (Q_SCALE * Q_SCALE)

_CACHE = {}


def _build_cova_nc():
    """Per-core: x2 pair [2, 8, 128, 1024] f32 -> cova pair [2, 128, 128] f32."""
    nc = bacc.Bacc("TRN2", target_bir_lowering=False, debug=False,
                   num_devices=N_CORES)
    x2p = nc.dram_tensor("x2p", [WS, S, C, HW], F32, kind="ExternalInput").ap()
    cov = nc.dram_tensor("cova_pair", [WS, C, C], F32, kind="ExternalOutput").ap()

    inv_nm1 = 1.0 / (NS - 1)
    # mean scale so that mmT comes out as N/(N-1) * m m^T directly:
    # m'' = msum * sqrt(N/(N-1)) / N
    mscale = float(np.sqrt(NS / (NS - 1.0)) / NS)

    with tile.TileContext(nc) as tc:
        NK = HW // 128  # 128-column transpose chunks per shot

        with (
            tc.tile_pool(name="consts", bufs=1) as consts,
            tc.tile_pool(name="raw", bufs=3) as raw,
            tc.tile_pool(name="xb", bufs=9) as xbp,
            tc.tile_pool(name="xt", bufs=3) as xtp,
            tc.tile_pool(name="small", bufs=4) as small,
            tc.tile_pool(name="scratch", bufs=2) as scratch,
            tc.tile_pool(name="cout", bufs=2) as cout,
            tc.tile_pool(name="pt", bufs=2, space="PSUM") as pt,
            tc.tile_pool(name="pe", bufs=2, space="PSUM") as pe,
            tc.tile_pool(name="pm", bufs=1, space="PSUM") as pm,
            tc.tile_pool(name="pmr", bufs=1, space="PSUM") as pmr,
        ):
            ident = consts.tile([128, 128], BF16)
            make_identity(nc, ident)
            ident32 = consts.tile([128, 128], F32)
            make_identity(nc, ident32)

            for w in range(WS):
                # load shots (split across the SP HWDGE queue and the gpsimd
                # SWDGE queue so transfers overlap), convert to bf16 on ACT
                # while accumulating the per-channel sum into msum columns
                xb16 = []
                msum = small.tile([C, S], F32)
                for s in range(S):
                    xr = raw.tile([C, HW], F32, tag="raw")
                    eng = nc.default_dma_engine if s % 2 == 0 else nc.gpsimd
                    eng.dma_start(out=xr, in_=x2p[w, s])
                    xb = xbp.tile([C, HW], BF16, tag=f"xb{s}")
                    nc.scalar.activation(xb, xr, mybir.ActivationFunctionType.Copy,
                                         accum_out=msum[:, s : s + 1])
                    xb16.append(xb)

                # m'' = (sum_s msum[:, s]) * mscale, as fp32 [C, 1]
                mtot = small.tile([C, 1], F32)
                nc.vector.reduce_sum(mtot, msum, axis=mybir.AxisListType.X)
                mpp = small.tile([C, 1], F32)
                nc.scalar.mul(mpp, mtot, mscale)

                # E = sum over 64 chunks of X_chunk X_chunk^T  (bf16, fp32
                # acc). Transposes land batched in one psum bank so a single
                # DVE copy stages all 8 chunks of a shot.
                e_ps = pe.tile([C, C], F32, tag="E")
                for s in range(S):
                    xt_ps = pt.tile([128, NK, C], BF16, tag="xt_ps")
                    for k in range(NK):
                        chunk = xb16[s][:, k * 128 : (k + 1) * 128]
                        nc.tensor.transpose(xt_ps[:, k, :], chunk, ident)
                    xt = xtp.tile([128, NK, C], BF16, tag="xt")
                    nc.vector.tensor_copy(xt, xt_ps)
                    for k in range(NK):
                        nc.tensor.matmul(e_ps, xt[:, k, :], xt[:, k, :],
                                         start=(s == 0 and k == 0),
                                         stop=(s == S - 1 and k == NK - 1))

                # mmT = m'' m''^T via K=1 matmul; needs m'' as a [1, C] row
                mrow_ps = pmr.tile([1, C], F32, tag="mrow")
                nc.tensor.transpose(mrow_ps, mpp, ident32)
                mrow = small.tile([1, C], F32)
                nc.vector.tensor_copy(mrow, mrow_ps)
                mmT_ps = pm.tile([C, C], F32, tag="mmT")
                nc.tensor.matmul(mmT_ps, mrow, mrow, start=True, stop=True)
                mmT = scratch.tile([C, C], F32, tag="mmT_sb")
                nc.vector.tensor_copy(mmT, mmT_ps)

                # C_w = E * 1/(N-1) - mmT
                cw = cout.tile([C, C], F32, tag="cw")
                nc.vector.scalar_tensor_tensor(
                    out=cw, in0=e_ps, scalar=inv_nm1, in1=mmT,
                    op0=mybir.AluOpType.mult, op1=mybir.AluOpType.subtract)
                nc.default_dma_engine.dma_start(out=cov[w], in_=cw)

    nc.compile()
    return nc


# classes on the ACT square path (mats = cholesky factor); odd classes use
# the DVE multiply path (mats = cova). Strict alternation keeps both engines
# fed every other class — any run of same-engine classes stalls the other.
ACT_SET = frozenset(range(0, W, 2))


def _build_sim_nc():
    """Per-core: x1 shard [32, 128, 1024] + mats [16, 128, 128] bf16 ->
    sim shard [32, 16, 1024]."""
    nc = bacc.Bacc("TRN2", target_bir_lowering=False, debug=False,
                   num_devices=N_CORES)
    x1s = nc.dram_tensor("x1s", [BS, C, HW], F32, kind="ExternalInput").ap()
    mats = nc.dram_tensor("mats", [W, C, C], BF16, kind="ExternalInput").ap()
    out = nc.dram_tensor("sim", [BS, W, HW], F32, kind="ExternalOutput").ap()

    NP = W // 2  # DoubleRow class pairs
    H = HW // 2  # 512-column halves

    with tile.TileContext(nc) as tc:
        with (
            tc.tile_pool(name="consts", bufs=1) as consts,
            tc.tile_pool(name="xb", bufs=4) as xbp,
            tc.tile_pool(name="sq", bufs=3) as sqp,
            tc.tile_pool(name="qn", bufs=3) as qnp,
            tc.tile_pool(name="vpk", bufs=2) as vpkp,
            tc.tile_pool(name="stg", bufs=2) as stgp,
            tc.tile_pool(name="small", bufs=9) as small,
            tc.tile_pool(name="pU", bufs=3, space="PSUM") as pU,
            tc.tile_pool(name="pacc", bufs=1, space="PSUM") as pacc,
        ):
            # one-hot DoubleRow reduction stationaries: pair p computes
            # rows (2p, 2p+1) of the sim accumulator from the packed
            # [128, 2, 512] fp8 V slice.
            drw = consts.tile([C, NP, 2, W], F8)
            nc.gpsimd.memset(drw, 0.0)
            for p in range(NP):
                for i in range(2):
                    w = 2 * p + i
                    nc.gpsimd.memset(drw[:, p, i, w : w + 1], 1.0)

            xbt = {}

            def emit_xb_dma(b):
                if b >= BS:
                    return
                xbt[b] = xbp.tile([C, HW], F32, tag="xb", name=f"xb{b}")
                nc.default_dma_engine.dma_start(out=xbt[b], in_=x1s[b])

            emit_xb_dma(0)

            # stationary matrices, already bf16 from the host: one DMA
            # (2048 x 256B descriptors) so the ACT sequencer stays free
            mat = consts.tile([C, W, C], BF16)
            nc.default_dma_engine.dma_start(
                out=mat, in_=mats.rearrange("w c d -> c w d"))

            emit_xb_dma(1)
            emit_xb_dma(2)

            qnt = {}

            nrmt = {}

            def emit_chain_act(b):
                """ACT half of the norm chain for query b: n2 -> nrm."""
                if b >= BS:
                    return
                sq = sqp.tile([C, HW], BF16, tag="sq")
                n2 = small.tile([C, 1], F32, tag="n2")
                nc.scalar.activation(sq, xbt[b],
                                     mybir.ActivationFunctionType.Square,
                                     accum_out=n2)
                nrmt[b] = small.tile([C, 1], F32, tag="nrm", name=f"nrm{b}")
                nc.scalar.sqrt(nrmt[b], n2)

            def emit_chain_rest(b):
                """DVE recip + Pool qn. Emitted mid-query so the DVE FIFO
                head never blocks on the cross-engine sqrt dependency."""
                if b >= BS:
                    return
                rinv = small.tile([C, 1], F32, tag="rinv")
                nc.vector.reciprocal(rinv, nrmt.pop(b))
                qnt[b] = qnp.tile([C, HW], BF16, tag="qn", name=f"qn{b}")
                nc.gpsimd.tensor_scalar(out=qnt[b], in0=xbt[b], scalar1=rinv,
                                        scalar2=Q_SCALE,
                                        op0=mybir.AluOpType.mult,
                                        op1=mybir.AluOpType.mult)

            def emit_stage(b, acc):
                """Stage + descale + output DMA for a finished query."""
                stg = stgp.tile([W, HW], F32, tag="stg")
                nc.scalar.activation(stg[:, 0:H], acc[:, 0:H],
                                     mybir.ActivationFunctionType.Copy,
                                     scale=DESCALE)
                nc.vector.tensor_scalar_mul(stg[:, H:HW], acc[:, H:HW], DESCALE)
                nc.default_dma_engine.dma_start(out=out[b], in_=stg)

            # two-query-ahead norm chains keep qn off every critical path
            emit_chain_act(0)
            emit_chain_rest(0)
            emit_chain_act(1)
            emit_chain_rest(1)

            def emit_drs(vpk, acc, h):
                """8 DoubleRow matmuls accumulate 16 class sums for one
                512-column half into the [16, 1024] psum tile."""
                cols = slice(h * H, (h + 1) * H)
                for p in range(NP):
                    nc.tensor.matmul(
                        acc[:, cols], drw[:, p, :, :], vpk[:, p, :, cols],
                        start=(p == 0), stop=(p == NP - 1),
                        perf_mode=mybir.MatmulPerfMode.DoubleRow)

            prev = None  # (b, vpk) of the previous query
            for b in range(BS):
                emit_xb_dma(b + 3)
                emit_chain_act(b + 2)

                qn = qnt.pop(b)
                xbt.pop(b)
                vpk = vpkp.tile([C, NP, 2, HW], F8, tag="vpk")
                for w in range(W):
                    u_ps = pU.tile([C, HW], F32, tag="u")
                    for h in range(2):
                        cols = slice(h * H, (h + 1) * H)
                        nc.tensor.matmul(u_ps[:, cols], mat[:, w, :],
                                         qn[:, cols], start=True, stop=True)
                    vslot = vpk[:, w // 2, w % 2, :]
                    if w in ACT_SET:
                        # cholesky path: V = U^2 (ACT)
                        nc.scalar.square(vslot, u_ps)
                    else:
                        # direct path: V = U * qn (DVE)
                        nc.vector.tensor_mul(vslot, u_ps, qn)
                    # previous query's reduction + stage, deferred past the
                    # first mains of this query (in two half-bursts) so the
                    # engines have fresh U tiles when the old V work drains
                    if prev is not None:
                        pb, pvpk, pacc_t, pstg = prev
                        if w == 2:
                            emit_drs(pvpk, pacc_t, 0)
                            nc.scalar.activation(
                                pstg[:, 0:H], pacc_t[:, 0:H],
                                mybir.ActivationFunctionType.Copy,
                                scale=DESCALE)
                        elif w == 4:
                            emit_drs(pvpk, pacc_t, 1)
                            nc.vector.tensor_scalar_mul(
                                pstg[:, H:HW], pacc_t[:, H:HW], DESCALE)
                            nc.default_dma_engine.dma_start(
                                out=out[pb], in_=pstg)
                    if w == 3:
                        emit_chain_rest(b + 2)

                acc = pacc.tile([W, HW], F32, tag="acc")
                stg = stgp.tile([W, HW], F32, tag="stg")
                prev = (b, vpk, acc, stg)

            pb, pvpk, pacc_t, pstg = prev
            emit_drs(pvpk, pacc_t, 0)
            nc.scalar.activation(pstg[:, 0:H], pacc_t[:, 0:H],
                                 mybir.ActivationFunctionType.Copy,
                                 scale=DESCALE)
            emit_drs(pvpk, pacc_t, 1)
            nc.vector.tensor_scalar_mul(pstg[:, H:HW], pacc_t[:, H:HW], DESCALE)
            nc.default_dma_engine.dma_start(out=out[pb], in_=pstg)

    nc.compile()
    return nc


def kernel(x1: np.ndarray, x2: np.ndarray) -> np.ndarray:
    x1 = np.ascontiguousarray(np.asarray(x1, dtype=np.float32)).reshape(B, C, HW)
    x2 = np.ascontiguousarray(np.asarray(x2, dtype=np.float32)).reshape(W, S, C, HW)
    core_ids = list(range(N_CORES))

    if "cova" not in _CACHE:
        _CACHE["cova"] = _build_cova_nc()
    cova_in = [{"x2p": np.ascontiguousarray(x2[WS * k : WS * (k + 1)])}
               for k in range(N_CORES)]
    res1 = run_bass_kernel_spmd(_CACHE["cova"], cova_in, core_ids)
    cova = np.concatenate([res1.results[k]["cova_pair"] for k in range(N_CORES)], 0)

    # ACT_SET classes go through the cholesky/ACT-square path
    import ml_dtypes

    act_idx = sorted(ACT_SET)
    mats = cova.copy()
    mats[act_idx] = np.linalg.cholesky(
        cova[act_idx].astype(np.float64)).astype(np.float32)
    mats = np.ascontiguousarray(mats.astype(ml_dtypes.bfloat16))

    if "sim" not in _CACHE:
        _CACHE["sim"] = _build_sim_nc()
    sim_in = [{"x1s": np.ascontiguousarray(x1[BS * k : BS * (k + 1)]),
               "mats": mats} for k in range(N_CORES)]
    res2 = run_bass_kernel_spmd(_CACHE["sim"], sim_in, core_ids)
    sim = np.concatenate([res2.results[k]["sim"] for k in range(N_CORES)], 0)
    return sim.reshape(B, W * HW)
